# revision 15
# baseline (speedup 1.0000x reference)
"""TRN2 Bass kernel for nn_DeformableTransformer (deformable 1D encoder).

Sharding: 8 cores; core c owns 512 contiguous tokens of batch b=c//4
(quarter q4=c%4). Each core processes a 640-token slice (64-token halo
per side, zero-padded at sequence edges) uniformly through all 4
layers. After layer i, slice rows [16(i+1), 640-16(i+1)) are exact, so
the final center 512 rows are exact. No inter-core communication.

Layout: activations transposed in SBUF — x^T with D on partitions (16
k-tiles of (128 x 640)), dtype float32r (TF32; full PE rate at N>=256),
fp32 PSUM accumulation.

Deformable attention: offsets are small, so gather + lerp + attention
weighting collapses into a banded matrix per (head, 96-query tile):
A[q, j] = sum_p attn[q,h,p] * relu(1 - |q + off[q,h,p] - j|) over a
128-row value window [96g-16, 96g+112). A is built via iota + ACT relu
with per-partition bias + DVE min, PE-transposed, then applied as
matmuls against value windows (computed per 512-channel chunk).
Out-of-sequence rows are handled by zeroing value window rows.

Biases ride as K=1 ones-row matmuls inside each PSUM accumulation.
LayerNorm: ones-column matmul reductions over partitions + K=1
broadcast matmuls, chunked over 320-row groups.
"""
import sys
import os

sys.path.insert(0, '/opt/trn_rl_repo')

import numpy as np

import concourse.bass as bass  # noqa: F401
import concourse.tile as tile
from concourse import bacc, mybir
from concourse.bass_utils import run_bass_kernel_spmd

F32 = mybir.dt.float32
F32R = mybir.dt.float32r
ALU = mybir.AluOpType
ACT = mybir.ActivationFunctionType

D = 2048
H = 8
P = 4
DFF = 2048
BS = 2
T = 2048
NL = int(os.environ.get("KERN_NL", "4"))
DEBUG = bool(int(os.environ.get("KERN_DEBUG", "0")))
REPS = int(os.environ.get("KERN_REPS", "1"))

R = 640          # slice rows per core (512 + 2*64 halo)
KT = D // 128    # 16 k-tiles
QT = 96          # query tile
NQ = 7           # q-tiles per slice
CH = 320         # row chunk (640 = 2*320)
RP = 704         # padded free width (16 zero cols left, 48 right)
O0 = 16          # offset of row 0 in padded coords
HB = 16          # band halo


def _nq(g):
    return min(QT, R - QT * g)


def _layer_norm(nc, sb, psum, x, lnt, onesc, onesr, eps, gcol, bcol):
    """In-place LN over D (partition dim across KT tiles), per row chunk."""
    for c in range(2):
        cs = slice(O0 + c * CH, O0 + (c + 1) * CH)
        s1 = psum.tile([1, CH], F32, tag="psL", bufs=1)
        s2 = psum.tile([1, CH], F32, tag="psL2", bufs=1)
        for k in range(KT):
            sq = sb.tile([128, CH], F32R, tag="sq")
            nc.scalar.activation(sq[:], x[:, k, cs], ACT.Square)
            nc.tensor.matmul(s1[:], onesc[:], x[:, k, cs],
                             start=(k == 0), stop=(k == KT - 1))
            nc.tensor.matmul(s2[:], onesc[:], sq[:],
                             start=(k == 0), stop=(k == KT - 1))
        mu = sb.tile([1, CH], F32R, tag="mu")
        nc.scalar.activation(mu[:], s1[:], ACT.Copy, scale=1.0 / D)
        ex2 = sb.tile([1, CH], F32, tag="ex2")
        nc.scalar.activation(ex2[:], s2[:], ACT.Copy, scale=1.0 / D)
        musq = sb.tile([1, CH], F32, tag="musq")
        nc.vector.tensor_mul(musq[:], mu[:].bitcast(F32), mu[:].bitcast(F32))
        var = sb.tile([1, CH], F32, tag="var")
        nc.vector.tensor_sub(var[:], ex2[:], musq[:])
        sd = sb.tile([1, CH], F32, tag="sd")
        nc.scalar.activation(sd[:], var[:], ACT.Sqrt, bias=eps[0:1, 0:1])
        istd = sb.tile([1, CH], F32R, tag="istd")
        nc.vector.reciprocal(istd[:], sd[:])
        bmu = psum.tile([128, CH], F32, tag="psL", bufs=1)
        bis = psum.tile([128, CH], F32, tag="psL2", bufs=1)
        nc.tensor.matmul(bmu[:], onesr[0:1, 0:128], mu[:], start=True, stop=True)
        nc.tensor.matmul(bis[:], onesr[0:1, 0:128], istd[:], start=True, stop=True)
        for k in range(KT):
            t1 = sb.tile([128, CH], F32, tag="lnt1")
            nc.vector.tensor_sub(t1[:], x[:, k, cs].bitcast(F32), bmu[:])
            nc.vector.tensor_mul(t1[:], t1[:], bis[:])
            nc.vector.tensor_scalar(
                x[:, k, cs], t1[:], lnt[:, gcol + k:gcol + k + 1],
                lnt[:, bcol + k:bcol + k + 1], op0=ALU.mult, op1=ALU.add)


def _emit_layer(nc, sb, st, wst, psum, cfg, li):
    x = cfg['x']; samp = cfg['samp']; h1 = cfg['h1']; at_all = cfg['at']
    iota = cfg['iota']; eye = cfg['eye']
    onesr = cfg['onesr']; onesc = cfg['onesc']
    ramp17 = cfg['ramp17']; rampm15 = cfg['rampm15']
    vwin = cfg['vwin']; lnt = cfg['lnt']

    # ---- 1. off/attn projection into one PSUM bank ----
    woa = sb.tile([128, KT, 64], F32R, tag="woa", bufs=1)
    nc.sync.dma_start(woa[:], cfg['d_Woa'].ap()[li])
    boa = sb.tile([1, 64], F32R, tag="boa", bufs=1)
    nc.sync.dma_start(boa[:], cfg['d_boa'].ap()[li])
    ps_off = psum.tile([96, NQ * 64], F32, tag="psA")
    for k in range(KT):
        q_k = sb.tile([128, R], F32R, tag="lvl")
        nc.sync.dma_start(q_k[:], cfg['d_lvl'].ap()[k])
        nc.vector.tensor_add(q_k[:], q_k[:], x[:, k, O0:O0 + R])
        for g in range(NQ):
            qs = QT * g
            nc.tensor.matmul(ps_off[0:_nq(g), g * 64:(g + 1) * 64],
                             q_k[:, qs:qs + _nq(g)], woa[:, k, :],
                             start=(k == 0 and g == 0), stop=False)
    for g in range(NQ):
        nc.tensor.matmul(ps_off[0:_nq(g), g * 64:(g + 1) * 64],
                         onesr[0:1, 0:_nq(g)], boa[:],
                         start=False, stop=(g == NQ - 1))

    # ---- softmax over P; off/attn -> SBUF ----
    off_t = sb.tile([96, NQ, 32], F32, tag="off")
    attn_t = sb.tile([96, NQ, 32], F32, tag="attn")
    for g in range(NQ):
        nq = _nq(g)
        nc.vector.tensor_copy(off_t[0:nq, g, :],
                              ps_off[0:nq, g * 64:g * 64 + 32])
        e_t = sb.tile([96, 32], F32, tag="exp")
        nc.scalar.activation(e_t[0:nq, :],
                             ps_off[0:nq, g * 64 + 32:g * 64 + 64], ACT.Exp)
        s_t = sb.tile([96, 8], F32, tag="ssum")
        nc.vector.tensor_reduce(
            s_t[0:nq, :], e_t[0:nq, :].rearrange("q (h p) -> q h p", p=P),
            axis=mybir.AxisListType.X, op=ALU.add)
        r_t = sb.tile([96, 8], F32, tag="srec")
        nc.vector.reciprocal(r_t[0:nq, :], s_t[0:nq, :])
        rb = r_t[0:nq, :].rearrange("q (h o) -> q h o", o=1) \
            .broadcast_to((nq, 8, P))
        nc.vector.tensor_tensor(
            attn_t[0:nq, g, :].rearrange("q (h p) -> q h p", p=P),
            e_t[0:nq, :].rearrange("q (h p) -> q h p", p=P),
            rb, op=ALU.mult)

    # ---- 2. banded A construction + PE transpose -> at_all[g*H+h] ----
    for g in range(NQ):
        nq = _nq(g)
        u1 = sb.tile([96, 32], F32, tag="u1")
        nc.vector.tensor_scalar(u1[0:nq, :], off_t[0:nq, g, :],
                                ramp17[0:nq, 0:1], None, op0=ALU.add)
        u2 = sb.tile([96, 32], F32, tag="u2")
        nc.vector.tensor_scalar(u2[0:nq, :], off_t[0:nq, g, :], -1.0,
                                rampm15[0:nq, 0:1],
                                op0=ALU.mult, op1=ALU.add)
        for h in range(H):
            acc = None
            for p in range(P):
                c = h * P + p
                a_t = sb.tile([96, 128], F32, tag="hata")
                nc.scalar.activation(a_t[0:nq, :], iota[0:nq, :], ACT.Relu,
                                     bias=u1[0:nq, c:c + 1], scale=-1.0)
                b_t = sb.tile([96, 128], F32, tag="hatb")
                nc.scalar.activation(b_t[0:nq, :], iota[0:nq, :], ACT.Relu,
                                     bias=u2[0:nq, c:c + 1], scale=1.0)
                m_t = sb.tile([96, 128], F32, tag="hatm")
                nc.vector.tensor_tensor(m_t[0:nq, :], a_t[0:nq, :],
                                        b_t[0:nq, :], op=ALU.min)
                nacc = sb.tile([96, 128], F32, tag="Aacc")
                if acc is None:
                    nc.vector.tensor_scalar(
                        nacc[0:nq, :], m_t[0:nq, :],
                        attn_t[0:nq, g, c:c + 1], None, op0=ALU.mult)
                else:
                    nc.vector.scalar_tensor_tensor(
                        nacc[0:nq, :], m_t[0:nq, :],
                        attn_t[0:nq, g, c:c + 1], acc[0:nq, :],
                        op0=ALU.mult, op1=ALU.add)
                acc = nacc
            ps_tr = psum.tile([128, 96], F32, tag="psA")
            nc.tensor.transpose(ps_tr[:, 0:nq], acc[0:nq, :],
                                eye[0:nq, 0:nq])
            nc.vector.tensor_copy(at_all[:, g * H + h, 0:nq],
                                  ps_tr[:, 0:nq])

    # ---- 3. value windows + A@V, per 512-channel chunk ----
    bv_t = sb.tile([1, D], F32R, tag="bbig", bufs=1)
    nc.sync.dma_start(bv_t[:], cfg['d_bv'].ap()[li])
    for n in range(8):
        wv_n = wst.tile([128, KT, 256], F32R, tag="wld")
        nc.sync.dma_start(
            wv_n[:], cfg['d_Wv'].ap()[li, :, n * 256:(n + 1) * 256]
            .rearrange("(k p) m -> p k m", p=128))
        for g in range(NQ):
            ps_v = psum.tile([128, 256], F32, tag="psV")
            for k in range(KT):
                nc.tensor.matmul(ps_v[:], x[:, k, QT * g:QT * g + 128],
                                 wv_n[:, k, :], start=(k == 0), stop=False)
            nc.tensor.matmul(ps_v[:], onesr[0:1, 0:128],
                             bv_t[:, n * 256:(n + 1) * 256],
                             start=False, stop=True)
            win = sb.tile([128, 256], F32R, tag="win")
            nc.vector.tensor_scalar(win[:], ps_v[:],
                                    vwin[:, g:g + 1], None, op0=ALU.mult)
            qs, nq = QT * g, _nq(g)
            for hh in range(2):
                kc = n * 2 + hh
                ps_s = psum.tile([128, 96], F32, tag="psA")
                nc.tensor.matmul(ps_s[:, 0:nq],
                                 win[:, hh * 128:(hh + 1) * 128],
                                 at_all[:, g * H + kc // 2, 0:nq],
                                 start=True, stop=True)
                nc.vector.tensor_copy(samp[:, kc, qs:qs + nq], ps_s[:, 0:nq])

    if DEBUG and li == 0:
        for k in range(KT):
            nc.sync.dma_start(cfg['dbg_samp'].ap()[k],
                              samp[:, k, :].bitcast(F32))
        nc.sync.dma_start(cfg['dbg_off'].ap(), off_t[:])
        nc.sync.dma_start(cfg['dbg_attn'].ap(), attn_t[:])

    # ---- 4. Wout + residual; LN1 ----
    bout_c = sb.tile([128, KT], F32, tag="bcol", bufs=1)
    nc.sync.dma_start(bout_c[:], cfg['d_boutc'].ap()[li])
    for m in range(KT):
        w_m = wst.tile([128, KT, 128], F32R, tag="wld", padded_shape=[128, KT, 256])
        nc.sync.dma_start(
            w_m[:], cfg['d_Wout'].ap()[li, :, m * 128:(m + 1) * 128]
            .rearrange("(k p) m -> p k m", p=128))
        for c in range(2):
            cs = slice(c * CH, (c + 1) * CH)
            ps = psum.tile([128, CH], F32, tag="psB")
            for k in range(KT):
                nc.tensor.matmul(ps[:], w_m[:, k, :], samp[:, k, cs],
                                 start=(k == 0), stop=(k == KT - 1))
            nc.vector.scalar_tensor_tensor(
                x[:, m, O0 + c * CH:O0 + (c + 1) * CH], ps[:],
                bout_c[:, m:m + 1], x[:, m, O0 + c * CH:O0 + (c + 1) * CH],
                op0=ALU.add, op1=ALU.add)
    _layer_norm(nc, sb, psum, x, lnt, onesc, onesr, cfg['eps'],
                li * KT, (NL + li) * KT)

    # ---- 5. FFN (row-chunked) + residual; LN2 ----
    b1_c = sb.tile([128, KT], F32, tag="bcol1", bufs=1)
    nc.sync.dma_start(b1_c[:], cfg['d_b1c'].ap()[li])
    b2_c = sb.tile([128, KT], F32, tag="bcol2", bufs=1)
    nc.sync.dma_start(b2_c[:], cfg['d_b2c'].ap()[li])
    for c in range(2):
        cs = slice(c * CH, (c + 1) * CH)
        for mf in range(KT):
            w_m = wst.tile([128, KT, 128], F32R, tag="wld", padded_shape=[128, KT, 256])
            nc.sync.dma_start(
                w_m[:], cfg['d_W1'].ap()[li, :, mf * 128:(mf + 1) * 128]
                .rearrange("(k p) m -> p k m", p=128))
            ps = psum.tile([128, CH], F32, tag="psB")
            for k in range(KT):
                nc.tensor.matmul(ps[:], w_m[:, k, :],
                                 x[:, k, O0 + c * CH:O0 + (c + 1) * CH],
                                 start=(k == 0), stop=(k == KT - 1))
            nc.scalar.activation(h1[:, mf, :], ps[:], ACT.Relu,
                                 bias=b1_c[:, mf:mf + 1])
        for m in range(KT):
            w_m = wst.tile([128, KT, 128], F32R, tag="wld", padded_shape=[128, KT, 256])
            nc.sync.dma_start(
                w_m[:], cfg['d_W2'].ap()[li, :, m * 128:(m + 1) * 128]
                .rearrange("(k p) m -> p k m", p=128))
            ps = psum.tile([128, CH], F32, tag="psB")
            for k in range(KT):
                nc.tensor.matmul(ps[:], w_m[:, k, :], h1[:, k, :],
                                 start=(k == 0), stop=(k == KT - 1))
            nc.vector.scalar_tensor_tensor(
                x[:, m, O0 + c * CH:O0 + (c + 1) * CH], ps[:],
                b2_c[:, m:m + 1], x[:, m, O0 + c * CH:O0 + (c + 1) * CH],
                op0=ALU.add, op1=ALU.add)
    _layer_norm(nc, sb, psum, x, lnt, onesc, onesr, cfg['eps'],
                (2 * NL + li) * KT, (3 * NL + li) * KT)


def build_program():
    nc = bacc.Bacc("TRN2", target_bir_lowering=False, debug=False,
                   num_devices=8)
    cfg = {}
    cfg['d_src'] = nc.dram_tensor("src_sl", [KT, 128, RP], F32R,
                                  kind="ExternalInput")
    cfg['d_lvl'] = nc.dram_tensor("lvl_sl", [KT, 128, R], F32R,
                                  kind="ExternalInput")
    cfg['d_vwin'] = nc.dram_tensor("vwin", [128, NQ], F32,
                                   kind="ExternalInput")
    cfg['d_iota'] = nc.dram_tensor("iota2d", [128, 128], F32,
                                   kind="ExternalInput")
    cfg['d_eye'] = nc.dram_tensor("eye", [128, 128], F32,
                                  kind="ExternalInput")
    cfg['d_ramp17'] = nc.dram_tensor("ramp17", [128, 1], F32,
                                     kind="ExternalInput")
    cfg['d_rampm15'] = nc.dram_tensor("rampm15", [128, 1], F32,
                                      kind="ExternalInput")
    cfg['d_lnt'] = nc.dram_tensor("lnt", [128, 4 * NL * KT], F32,
                                  kind="ExternalInput")
    cfg['d_onesr'] = nc.dram_tensor("onesr_in", [1, 128], F32R,
                                    kind="ExternalInput")
    cfg['d_onesc'] = nc.dram_tensor("onesc_in", [128, 1], F32R,
                                    kind="ExternalInput")
    for nm, shp in [('Woa', [NL, 128, KT, 64]), ('boa', [NL, 1, 64]),
                    ('Wv', [NL, D, D]), ('bv', [NL, 1, D]),
                    ('Wout', [NL, D, D]),
                    ('W1', [NL, D, DFF]),
                    ('W2', [NL, DFF, D])]:
        cfg['d_' + nm] = nc.dram_tensor(nm, shp, F32R, kind="ExternalInput")
    for nm in ['boutc', 'b1c', 'b2c']:
        cfg['d_' + nm] = nc.dram_tensor(nm, [NL, 128, KT], F32,
                                        kind="ExternalInput")
    d_out = nc.dram_tensor("xout", [KT, 128, 512], F32, kind="ExternalOutput")
    if DEBUG:
        cfg['dbg_samp'] = nc.dram_tensor("dbg_samp", [KT, 128, R], F32,
                                         kind="ExternalOutput")
        cfg['dbg_off'] = nc.dram_tensor("dbg_off", [96, NQ, 32], F32,
                                        kind="ExternalOutput")
        cfg['dbg_attn'] = nc.dram_tensor("dbg_attn", [96, NQ, 32], F32,
                                         kind="ExternalOutput")

    with tile.TileContext(nc) as tc:
        from contextlib import ExitStack
        with ExitStack() as ctx:
            sb = ctx.enter_context(tc.tile_pool(name="sb", bufs=2))
            st = ctx.enter_context(tc.tile_pool(name="st", bufs=1))
            wst = ctx.enter_context(tc.tile_pool(name="wst", bufs=2))
            psum = ctx.enter_context(
                tc.tile_pool(name="psum", bufs=2, space="PSUM"))

            x = st.tile([128, KT, RP], F32R, tag="x")
            for k in range(KT):
                nc.sync.dma_start(x[:, k, :], cfg['d_src'].ap()[k])
            cfg['x'] = x
            cfg['samp'] = st.tile([128, KT, R], F32R, tag="samp", name="samp")
            cfg['h1'] = st.tile([128, KT, CH], F32R, tag="h1", name="h1")
            cfg['at'] = st.tile([128, NQ * H, 96], F32R, tag="at", name="at")
            for nm in ['iota', 'eye', 'ramp17', 'rampm15', 'vwin', 'lnt']:
                tl = st.tile(cfg['d_' + nm].shape, F32, tag=nm, name=nm)
                nc.sync.dma_start(tl[:], cfg['d_' + nm].ap())
                cfg[nm] = tl
            onesr = st.tile([1, 128], F32R, tag="onesr")
            nc.sync.dma_start(onesr[:], cfg['d_onesr'].ap())
            cfg['onesr'] = onesr
            onesc = st.tile([128, 1], F32R, tag="onesc")
            nc.sync.dma_start(onesc[:], cfg['d_onesc'].ap())
            cfg['onesc'] = onesc
            eps = st.tile([1, 1], F32, tag="eps")
            nc.vector.memset(eps[:], 1e-5)
            cfg['eps'] = eps

            with nc.allow_low_precision(reason="tf32 kernel by design"):
                for _rep in range(REPS):
                    for li in range(NL):
                        _emit_layer(nc, sb, st, wst, psum, cfg, li)

            for k in range(KT):
                nc.sync.dma_start(d_out.ap()[k],
                                  x[:, k, O0 + 64:O0 + 576].bitcast(F32))
    nc.compile()
    return nc


_CACHE = {}


def _host_prep(inputs):
    src = np.ascontiguousarray(np.asarray(inputs['src'], np.float32))
    mask = np.asarray(inputs['mask'])
    emb = np.cumsum(mask.astype(np.float32), axis=1)
    emb = emb / (emb[:, -1:] + 1e-6) * np.float32(2 * np.pi)
    dim_t = (10000.0 ** (2.0 * (np.arange(D) // 2).astype(np.float32) / D)
             ).astype(np.float32)
    pos = emb[:, :, None] / dim_t
    posf = np.empty((BS, T, D), np.float32)
    posf[:, :, 0::2] = np.sin(pos[:, :, 0::2])
    posf[:, :, 1::2] = np.cos(pos[:, :, 1::2])
    lvl = posf + np.asarray(inputs['level_embed'], np.float32)  # (BS,T,D)

    idx = np.arange(128, dtype=np.float32)
    shared = {
        'iota2d': np.ascontiguousarray(np.broadcast_to(idx, (128, 128))),
        'eye': np.eye(128, dtype=np.float32),
        'ramp17': (idx + 17)[:, None].copy(),
        'rampm15': (-(idx + 15))[:, None].copy(),
        'onesr_in': np.ones((1, 128), np.float32),
        'onesc_in': np.ones((128, 1), np.float32),
    }
    lnt = np.zeros((128, 4 * NL * KT), np.float32)
    for nm, base in [('ln1_g', 0), ('ln1_b', NL * KT),
                     ('ln2_g', 2 * NL * KT), ('ln2_b', 3 * NL * KT)]:
        a = np.asarray(inputs[nm], np.float32)
        for i in range(NL):
            lnt[:, base + i * KT:base + (i + 1) * KT] = a[i].reshape(KT, 128).T
    shared['lnt'] = lnt
    woa = np.concatenate([np.asarray(inputs['Wo'], np.float32),
                          np.asarray(inputs['Wa'], np.float32)], axis=2)[:NL]
    shared['Woa'] = np.ascontiguousarray(
        woa.reshape(NL, KT, 128, 64).transpose(0, 2, 1, 3))
    shared['boa'] = np.ascontiguousarray(np.concatenate(
        [np.asarray(inputs['bo'], np.float32),
         np.asarray(inputs['ba'], np.float32)], axis=1)[:NL, None, :])
    for nm in ['Wv', 'Wout', 'W1', 'W2']:
        shared[nm] = np.ascontiguousarray(
            np.asarray(inputs[nm], np.float32)[:NL])
    shared['bv'] = np.ascontiguousarray(
        np.asarray(inputs['bv'], np.float32)[:NL, None, :])
    for src_nm, dst_nm in [('bout', 'boutc'), ('b1', 'b1c'), ('b2', 'b2c')]:
        a = np.asarray(inputs[src_nm], np.float32)[:NL]  # (NL, D)
        shared[dst_nm] = np.ascontiguousarray(
            a.reshape(NL, KT, 128).transpose(0, 2, 1))

    in_maps = []
    for c in range(8):
        b, q4 = c // 4, c % 4
        lo = 512 * q4 - 64
        s, e = max(lo, 0), min(lo + R, T)
        xs = np.zeros((D, RP), np.float32)
        xs[:, O0 + s - lo:O0 + e - lo] = src[b, :, s:e]
        lv = np.zeros((D, R), np.float32)
        lv[:, s - lo:e - lo] = lvl[b, s:e].T
        valid = np.zeros(R + 256, np.float32)
        valid[s - lo + 128:e - lo + 128] = 1.0
        vwin = np.zeros((128, NQ), np.float32)
        for g in range(NQ):
            wlo = QT * g - HB
            vwin[:, g] = valid[wlo + 128:wlo + 256]
        in_maps.append({**shared,
                        'src_sl': xs.reshape(KT, 128, RP),
                        'lvl_sl': lv.reshape(KT, 128, R),
                        'vwin': vwin})
    return in_maps


def kernel(**inputs):
    if 'nc' not in _CACHE:
        _CACHE['nc'] = build_program()
    nc = _CACHE['nc']
    in_maps = _host_prep(inputs)
    res = run_bass_kernel_spmd(nc, in_maps, list(range(8)))
    _CACHE['res'] = res
    out = np.zeros((BS, D, T), np.float32)
    for c in range(8):
        b, q4 = c // 4, c % 4
        out[b, :, 512 * q4:512 * (q4 + 1)] = \
            res.results[c]['xout'].reshape(D, 512)
    mask = np.asarray(inputs['mask'])
    return out, mask[:, None]


# revision 21
# speedup vs baseline: 1.4908x; 1.4908x over previous
"""TRN2 Bass kernel for nn_DeformableTransformer (deformable 1D encoder).

Sharding: 8 cores; core c owns 512 contiguous tokens of batch b=c//4
(quarter q4=c%4). Each core processes a 640-token slice (64-token halo
per side, zero-padded at sequence edges) uniformly through all 4
layers. After layer i, slice rows [16(i+1), 640-16(i+1)) are exact, so
the final center 512 rows are exact. No inter-core communication.

Layout: activations transposed in SBUF — x^T with D on partitions (16
k-tiles of (128 x 640)), dtype float32r (TF32; full PE rate at N>=256),
fp32 PSUM accumulation.

Deformable attention: offsets are small, so gather + lerp + attention
weighting collapses into a banded matrix per (head, 96-query tile):
A[q, j] = sum_p attn[q,h,p] * relu(1 - |q + off[q,h,p] - j|) over a
128-row value window [96g-16, 96g+112). A is built via iota + ACT relu
with per-partition bias + DVE min, PE-transposed, then applied as
matmuls against value windows (computed per 512-channel chunk).
Out-of-sequence rows are handled by zeroing value window rows.

Biases ride as K=1 ones-row matmuls inside each PSUM accumulation.
LayerNorm: ones-column matmul reductions over partitions + K=1
broadcast matmuls, chunked over 320-row groups.
"""
import sys
import os

sys.path.insert(0, '/opt/trn_rl_repo')

import numpy as np

import concourse.bass as bass  # noqa: F401
import concourse.tile as tile
from concourse import bacc, mybir
from concourse.bass_utils import run_bass_kernel_spmd

F32 = mybir.dt.float32
F32R = mybir.dt.float32r
ALU = mybir.AluOpType
ACT = mybir.ActivationFunctionType

D = 2048
H = 8
P = 4
DFF = 2048
BS = 2
T = 2048
NL = int(os.environ.get("KERN_NL", "4"))
DEBUG = bool(int(os.environ.get("KERN_DEBUG", "0")))
REPS = int(os.environ.get("KERN_REPS", "1"))

R = 640          # slice rows per core (512 + 2*64 halo)
KT = D // 128    # 16 k-tiles
QT = 96          # query tile
NQ = 7           # q-tiles per slice
CH = 320         # row chunk (640 = 2*320)
RP = 704         # padded free width (16 zero cols left, 48 right)
O0 = 16          # offset of row 0 in padded coords
HB = 16          # band halo


def _nq(g):
    return min(QT, R - QT * g)


def _layer_norm(nc, sb, psum, x, lnt, onesc, onesr, eps, gcol, bcol):
    """In-place LN over D (partition dim across KT tiles), per row chunk."""
    for c in range(2):
        cs = slice(O0 + c * CH, O0 + (c + 1) * CH)
        s1 = psum.tile([1, CH], F32, tag="psL", bufs=1)
        s2 = psum.tile([1, CH], F32, tag="psL2", bufs=1)
        for k in range(KT):
            sq = sb.tile([128, CH], F32R, tag="sq")
            nc.scalar.activation(sq[:], x[:, k, cs], ACT.Square)
            nc.tensor.matmul(s1[:], onesc[:], x[:, k, cs],
                             start=(k == 0), stop=(k == KT - 1))
            nc.tensor.matmul(s2[:], onesc[:], sq[:],
                             start=(k == 0), stop=(k == KT - 1))
        mu = sb.tile([1, CH], F32R, tag="mu")
        nc.scalar.activation(mu[:], s1[:], ACT.Copy, scale=1.0 / D)
        ex2 = sb.tile([1, CH], F32, tag="ex2")
        nc.scalar.activation(ex2[:], s2[:], ACT.Copy, scale=1.0 / D)
        musq = sb.tile([1, CH], F32, tag="musq")
        nc.vector.tensor_mul(musq[:], mu[:].bitcast(F32), mu[:].bitcast(F32))
        var = sb.tile([1, CH], F32, tag="var")
        nc.vector.tensor_sub(var[:], ex2[:], musq[:])
        sd = sb.tile([1, CH], F32, tag="sd")
        nc.scalar.activation(sd[:], var[:], ACT.Sqrt, bias=eps[0:1, 0:1])
        istd = sb.tile([1, CH], F32R, tag="istd")
        nc.vector.reciprocal(istd[:], sd[:])
        bmu = psum.tile([128, CH], F32, tag="psL", bufs=1)
        bis = psum.tile([128, CH], F32, tag="psL2", bufs=1)
        nc.tensor.matmul(bmu[:], onesr[0:1, 0:128], mu[:], start=True, stop=True)
        nc.tensor.matmul(bis[:], onesr[0:1, 0:128], istd[:], start=True, stop=True)
        for k in range(KT):
            t1 = sb.tile([128, CH], F32, tag="lnt1")
            nc.vector.tensor_sub(t1[:], x[:, k, cs].bitcast(F32), bmu[:])
            nc.vector.tensor_mul(t1[:], t1[:], bis[:])
            nc.vector.tensor_scalar(
                x[:, k, cs], t1[:], lnt[:, gcol + k:gcol + k + 1],
                lnt[:, bcol + k:bcol + k + 1], op0=ALU.mult, op1=ALU.add)


def _emit_layer(nc, sb, st, wst, psum, cfg, li):
    x = cfg['x']; samp = cfg['samp']; at_all = cfg['at']
    iota = cfg['iota']; eye = cfg['eye']
    onesr = cfg['onesr']; onesc = cfg['onesc']
    rampm16 = cfg['rampm16']
    vwin = cfg['vwin']; lnt = cfg['lnt']

    # ---- 1. off/attn projection (lvl@Woa + bias folded in on host) ----
    woa = sb.tile([128, KT, 64], F32R, tag="woa", bufs=1)
    nc.sync.dma_start(woa[:], cfg['d_Woa'].ap()[li])
    lvlog = sb.tile([96, NQ * 64], F32, tag="lvlog", bufs=1)
    nc.sync.dma_start(lvlog[:], cfg['d_lvlog'].ap()[li])
    ps_off = psum.tile([96, NQ * 64], F32, tag="psA")
    for k in range(KT):
        for g in range(NQ):
            qs = QT * g
            nc.tensor.matmul(ps_off[0:_nq(g), g * 64:(g + 1) * 64],
                             x[:, k, O0 + qs:O0 + qs + _nq(g)], woa[:, k, :],
                             start=(k == 0 and g == 0),
                             stop=(k == KT - 1 and g == NQ - 1))

    # ---- softmax over P; off/attn -> SBUF ----
    off_t = sb.tile([96, NQ, 32], F32, tag="off")
    attn_t = sb.tile([96, NQ, 32], F32, tag="attn")
    for g in range(NQ):
        nq = _nq(g)
        nc.vector.tensor_add(off_t[0:nq, g, :],
                             ps_off[0:nq, g * 64:g * 64 + 32],
                             lvlog[0:nq, g * 64:g * 64 + 32])
        e_t = sb.tile([96, 32], F32, tag="exp")
        al_t = sb.tile([96, 32], F32, tag="alog")
        nc.vector.tensor_add(al_t[0:nq, :],
                             ps_off[0:nq, g * 64 + 32:g * 64 + 64],
                             lvlog[0:nq, g * 64 + 32:g * 64 + 64])
        nc.scalar.activation(e_t[0:nq, :], al_t[0:nq, :], ACT.Exp)
        s_t = sb.tile([96, 8], F32, tag="ssum")
        nc.vector.tensor_reduce(
            s_t[0:nq, :], e_t[0:nq, :].rearrange("q (h p) -> q h p", p=P),
            axis=mybir.AxisListType.X, op=ALU.add)
        r_t = sb.tile([96, 8], F32, tag="srec")
        nc.vector.reciprocal(r_t[0:nq, :], s_t[0:nq, :])
        rb = r_t[0:nq, :].rearrange("q (h o) -> q h o", o=1) \
            .broadcast_to((nq, 8, P))
        nc.vector.tensor_tensor(
            attn_t[0:nq, g, :].rearrange("q (h p) -> q h p", p=P),
            e_t[0:nq, :].rearrange("q (h p) -> q h p", p=P),
            rb, op=ALU.mult)

    # ---- 2. banded A construction + PE transpose -> at_all[g*H+h] ----
    for g in range(NQ):
        nq = _nq(g)
        un = sb.tile([96, 32], F32, tag="un")
        nc.vector.tensor_scalar(un[0:nq, :], off_t[0:nq, g, :], -1.0,
                                rampm16[0:nq, 0:1],
                                op0=ALU.mult, op1=ALU.add)
        an = sb.tile([96, 32], F32, tag="an")
        nc.vector.tensor_scalar(an[0:nq, :], attn_t[0:nq, g, :], -1.0, None,
                                op0=ALU.mult)
        for h in range(H):
            acc = None
            for p in range(P):
                c = h * P + p
                d_t = sb.tile([96, 128], F32, tag="hata")
                nc.scalar.activation(d_t[0:nq, :], iota[0:nq, :], ACT.Abs,
                                     bias=un[0:nq, c:c + 1], scale=1.0)
                t_t = sb.tile([96, 128], F32, tag="hatm")
                if p % 2 == 0:
                    nc.vector.tensor_scalar(
                        t_t[0:nq, :], d_t[0:nq, :], an[0:nq, c:c + 1],
                        attn_t[0:nq, g, c:c + 1], op0=ALU.mult, op1=ALU.add)
                else:
                    nc.scalar.activation(
                        t_t[0:nq, :], d_t[0:nq, :], ACT.Identity,
                        bias=attn_t[0:nq, g, c:c + 1],
                        scale=an[0:nq, c:c + 1])
                nacc = sb.tile([96, 128], F32, tag="Aacc")
                if acc is None:
                    nc.vector.tensor_scalar(
                        nacc[0:nq, :], t_t[0:nq, :], 0.0, None, op0=ALU.max)
                else:
                    nc.vector.scalar_tensor_tensor(
                        nacc[0:nq, :], t_t[0:nq, :], 0.0, acc[0:nq, :],
                        op0=ALU.max, op1=ALU.add)
                acc = nacc
            ps_tr = psum.tile([128, 96], F32, tag="psA")
            nc.tensor.transpose(ps_tr[:, 0:nq], acc[0:nq, :],
                                eye[0:nq, 0:nq])
            nc.vector.tensor_copy(at_all[:, g * H + h, 0:nq],
                                  ps_tr[:, 0:nq])

    # ---- 3. value windows + A@V, per 512-channel chunk ----
    bv_t = sb.tile([1, D], F32R, tag="bbig", bufs=1)
    nc.sync.dma_start(bv_t[:], cfg['d_bv'].ap()[li])
    for n in range(8):
        wv_n = wst.tile([128, KT, 256], F32R, tag="wld")
        nc.sync.dma_start(
            wv_n[:], cfg['d_Wv'].ap()[li, :, n * 256:(n + 1) * 256]
            .rearrange("(k p) m -> p k m", p=128))
        for g in range(NQ):
            ps_v = psum.tile([128, 256], F32, tag="psV")
            for k in range(KT):
                nc.tensor.matmul(ps_v[:], x[:, k, QT * g:QT * g + 128],
                                 wv_n[:, k, :], start=(k == 0), stop=False)
            nc.tensor.matmul(ps_v[:], onesr[0:1, 0:128],
                             bv_t[:, n * 256:(n + 1) * 256],
                             start=False, stop=True)
            win = sb.tile([128, 256], F32R, tag="win", bufs=26)
            nc.vector.tensor_scalar(win[:], ps_v[:],
                                    vwin[:, g:g + 1], None, op0=ALU.mult)
            qs, nq = QT * g, _nq(g)
            ps_s = psum.tile([128, 192], F32, tag="psA")
            for hh in range(2):
                kc = n * 2 + hh
                nc.tensor.matmul(ps_s[:, hh * 96:hh * 96 + nq],
                                 win[:, hh * 128:(hh + 1) * 128],
                                 at_all[:, g * H + kc // 2, 0:nq],
                                 start=(hh == 0), stop=(hh == 1))
            nc.vector.tensor_copy(
                samp[:, n * 2:n * 2 + 2, qs:qs + nq],
                ps_s[:].rearrange("j (h q) -> j h q", h=2)[:, :, 0:nq])

    if DEBUG and li == 0:
        for k in range(KT):
            nc.sync.dma_start(cfg['dbg_samp'].ap()[k],
                              samp[:, k, :].bitcast(F32))
        nc.sync.dma_start(cfg['dbg_off'].ap(), off_t[:])
        nc.sync.dma_start(cfg['dbg_attn'].ap(), attn_t[:])

    # ---- 4. Wout + residual; LN1 ----
    bout_c = sb.tile([128, KT], F32, tag="bcol", bufs=1)
    nc.sync.dma_start(bout_c[:], cfg['d_boutc'].ap()[li])
    for m in range(KT):
        w_m = wst.tile([128, KT, 128], F32R, tag="wld", padded_shape=[128, KT, 256])
        nc.sync.dma_start(
            w_m[:], cfg['d_Wout'].ap()[li, :, m * 128:(m + 1) * 128]
            .rearrange("(k p) m -> p k m", p=128))
        for c in range(2):
            cs = slice(c * CH, (c + 1) * CH)
            ps = psum.tile([128, CH], F32, tag="psB")
            for k in range(KT):
                nc.tensor.matmul(ps[:], w_m[:, k, :], samp[:, k, cs],
                                 start=(k == 0), stop=(k == KT - 1))
            nc.vector.scalar_tensor_tensor(
                x[:, m, O0 + c * CH:O0 + (c + 1) * CH], ps[:],
                bout_c[:, m:m + 1], x[:, m, O0 + c * CH:O0 + (c + 1) * CH],
                op0=ALU.add, op1=ALU.add)
    _layer_norm(nc, sb, psum, x, lnt, onesc, onesr, cfg['eps'],
                li * KT, (NL + li) * KT)

    # ---- 5. FFN (row-chunked) + residual; LN2 ----
    b1_c = sb.tile([128, KT], F32, tag="bcol1", bufs=1)
    nc.sync.dma_start(b1_c[:], cfg['d_b1c'].ap()[li])
    b2_c = sb.tile([128, KT], F32, tag="bcol2", bufs=1)
    nc.sync.dma_start(b2_c[:], cfg['d_b2c'].ap()[li])
    h1 = samp  # aliased storage: samp is dead after Wout, reuse for h1
    for mf in range(KT):
        w_m = wst.tile([128, KT, 128], F32R, tag="wld", padded_shape=[128, KT, 256])
        nc.sync.dma_start(
            w_m[:], cfg['d_W1'].ap()[li, :, mf * 128:(mf + 1) * 128]
            .rearrange("(k p) m -> p k m", p=128))
        for c in range(2):
            ps = psum.tile([128, CH], F32, tag="psB")
            for k in range(KT):
                nc.tensor.matmul(ps[:], w_m[:, k, :],
                                 x[:, k, O0 + c * CH:O0 + (c + 1) * CH],
                                 start=(k == 0), stop=(k == KT - 1))
            nc.scalar.activation(h1[:, mf, c * CH:(c + 1) * CH], ps[:],
                                 ACT.Relu, bias=b1_c[:, mf:mf + 1])
    for m in range(KT):
        w_m = wst.tile([128, KT, 128], F32R, tag="wld", padded_shape=[128, KT, 256])
        nc.sync.dma_start(
            w_m[:], cfg['d_W2'].ap()[li, :, m * 128:(m + 1) * 128]
            .rearrange("(k p) m -> p k m", p=128))
        for c in range(2):
            ps = psum.tile([128, CH], F32, tag="psB")
            for k in range(KT):
                nc.tensor.matmul(ps[:], w_m[:, k, :],
                                 h1[:, k, c * CH:(c + 1) * CH],
                                 start=(k == 0), stop=(k == KT - 1))
            nc.vector.scalar_tensor_tensor(
                x[:, m, O0 + c * CH:O0 + (c + 1) * CH], ps[:],
                b2_c[:, m:m + 1], x[:, m, O0 + c * CH:O0 + (c + 1) * CH],
                op0=ALU.add, op1=ALU.add)
    _layer_norm(nc, sb, psum, x, lnt, onesc, onesr, cfg['eps'],
                (2 * NL + li) * KT, (3 * NL + li) * KT)


def build_program():
    nc = bacc.Bacc("TRN2", target_bir_lowering=False, debug=False,
                   num_devices=8)
    cfg = {}
    cfg['d_src'] = nc.dram_tensor("src_sl", [KT, 128, RP], F32R,
                                  kind="ExternalInput")
    cfg['d_lvlog'] = nc.dram_tensor("lvlog", [NL, 96, NQ * 64], F32,
                                    kind="ExternalInput")
    cfg['d_vwin'] = nc.dram_tensor("vwin", [128, NQ], F32,
                                   kind="ExternalInput")
    cfg['d_iota'] = nc.dram_tensor("iota2d", [128, 128], F32,
                                   kind="ExternalInput")
    cfg['d_eye'] = nc.dram_tensor("eye", [128, 128], F32,
                                  kind="ExternalInput")
    cfg['d_rampm16'] = nc.dram_tensor("rampm16", [128, 1], F32,
                                      kind="ExternalInput")
    cfg['d_lnt'] = nc.dram_tensor("lnt", [128, 4 * NL * KT], F32,
                                  kind="ExternalInput")
    cfg['d_onesr'] = nc.dram_tensor("onesr_in", [1, 128], F32R,
                                    kind="ExternalInput")
    cfg['d_onesc'] = nc.dram_tensor("onesc_in", [128, 1], F32R,
                                    kind="ExternalInput")
    for nm, shp in [('Woa', [NL, 128, KT, 64]),
                    ('Wv', [NL, D, D]), ('bv', [NL, 1, D]),
                    ('Wout', [NL, D, D]),
                    ('W1', [NL, D, DFF]),
                    ('W2', [NL, DFF, D])]:
        cfg['d_' + nm] = nc.dram_tensor(nm, shp, F32R, kind="ExternalInput")
    for nm in ['boutc', 'b1c', 'b2c']:
        cfg['d_' + nm] = nc.dram_tensor(nm, [NL, 128, KT], F32,
                                        kind="ExternalInput")
    d_out = nc.dram_tensor("xout", [KT, 128, 512], F32, kind="ExternalOutput")
    if DEBUG:
        cfg['dbg_samp'] = nc.dram_tensor("dbg_samp", [KT, 128, R], F32,
                                         kind="ExternalOutput")
        cfg['dbg_off'] = nc.dram_tensor("dbg_off", [96, NQ, 32], F32,
                                        kind="ExternalOutput")
        cfg['dbg_attn'] = nc.dram_tensor("dbg_attn", [96, NQ, 32], F32,
                                         kind="ExternalOutput")

    with tile.TileContext(nc) as tc:
        from contextlib import ExitStack
        with ExitStack() as ctx:
            sb = ctx.enter_context(tc.tile_pool(name="sb", bufs=2))
            st = ctx.enter_context(tc.tile_pool(name="st", bufs=1))
            wst = ctx.enter_context(tc.tile_pool(name="wst", bufs=2))
            psum = ctx.enter_context(
                tc.tile_pool(name="psum", bufs=2, space="PSUM"))

            x = st.tile([128, KT, RP], F32R, tag="x")
            for k in range(KT):
                nc.sync.dma_start(x[:, k, :], cfg['d_src'].ap()[k])
            cfg['x'] = x
            cfg['samp'] = st.tile([128, KT, R], F32R, tag="samp", name="samp")
            cfg['at'] = st.tile([128, NQ * H, 96], F32R, tag="at", name="at")
            for nm in ['iota', 'eye', 'rampm16', 'vwin', 'lnt']:
                tl = st.tile(cfg['d_' + nm].shape, F32, tag=nm, name=nm)
                nc.sync.dma_start(tl[:], cfg['d_' + nm].ap())
                cfg[nm] = tl
            onesr = st.tile([1, 128], F32R, tag="onesr")
            nc.sync.dma_start(onesr[:], cfg['d_onesr'].ap())
            cfg['onesr'] = onesr
            onesc = st.tile([128, 1], F32R, tag="onesc")
            nc.sync.dma_start(onesc[:], cfg['d_onesc'].ap())
            cfg['onesc'] = onesc
            eps = st.tile([1, 1], F32, tag="eps")
            nc.vector.memset(eps[:], 1e-5)
            cfg['eps'] = eps

            with nc.allow_low_precision(reason="tf32 kernel by design"):
                for _rep in range(REPS):
                    for li in range(NL):
                        _emit_layer(nc, sb, st, wst, psum, cfg, li)

            for k in range(KT):
                nc.sync.dma_start(d_out.ap()[k],
                                  x[:, k, O0 + 64:O0 + 576].bitcast(F32))
    nc.compile()
    return nc


_CACHE = {}


def _host_prep(inputs):
    src = np.ascontiguousarray(np.asarray(inputs['src'], np.float32))
    mask = np.asarray(inputs['mask'])
    emb = np.cumsum(mask.astype(np.float32), axis=1)
    emb = emb / (emb[:, -1:] + 1e-6) * np.float32(2 * np.pi)
    dim_t = (10000.0 ** (2.0 * (np.arange(D) // 2).astype(np.float32) / D)
             ).astype(np.float32)
    pos = emb[:, :, None] / dim_t
    posf = np.empty((BS, T, D), np.float32)
    posf[:, :, 0::2] = np.sin(pos[:, :, 0::2])
    posf[:, :, 1::2] = np.cos(pos[:, :, 1::2])
    lvl = posf + np.asarray(inputs['level_embed'], np.float32)  # (BS,T,D)

    idx = np.arange(128, dtype=np.float32)
    shared = {
        'iota2d': np.ascontiguousarray(np.broadcast_to(idx, (128, 128))),
        'eye': np.eye(128, dtype=np.float32),
        'rampm16': (-(idx + 16))[:, None].copy(),
        'onesr_in': np.ones((1, 128), np.float32),
        'onesc_in': np.ones((128, 1), np.float32),
    }
    lnt = np.zeros((128, 4 * NL * KT), np.float32)
    for nm, base in [('ln1_g', 0), ('ln1_b', NL * KT),
                     ('ln2_g', 2 * NL * KT), ('ln2_b', 3 * NL * KT)]:
        a = np.asarray(inputs[nm], np.float32)
        for i in range(NL):
            lnt[:, base + i * KT:base + (i + 1) * KT] = a[i].reshape(KT, 128).T
    shared['lnt'] = lnt
    woa_f = np.concatenate([np.asarray(inputs['Wo'], np.float32),
                            np.asarray(inputs['Wa'], np.float32)],
                           axis=2)[:NL]  # (NL, D, 64)
    shared['Woa'] = np.ascontiguousarray(
        woa_f.reshape(NL, KT, 128, 64).transpose(0, 2, 1, 3))
    boa = np.concatenate([np.asarray(inputs['bo'], np.float32),
                          np.asarray(inputs['ba'], np.float32)],
                         axis=1)[:NL]  # (NL, 64)
    for nm in ['Wv', 'Wout', 'W1', 'W2']:
        shared[nm] = np.ascontiguousarray(
            np.asarray(inputs[nm], np.float32)[:NL])
    shared['bv'] = np.ascontiguousarray(
        np.asarray(inputs['bv'], np.float32)[:NL, None, :])
    for src_nm, dst_nm in [('bout', 'boutc'), ('b1', 'b1c'), ('b2', 'b2c')]:
        a = np.asarray(inputs[src_nm], np.float32)[:NL]  # (NL, D)
        shared[dst_nm] = np.ascontiguousarray(
            a.reshape(NL, KT, 128).transpose(0, 2, 1))

    in_maps = []
    for c in range(8):
        b, q4 = c // 4, c % 4
        lo = 512 * q4 - 64
        s, e = max(lo, 0), min(lo + R, T)
        xs = np.zeros((D, RP), np.float32)
        xs[:, O0 + s - lo:O0 + e - lo] = src[b, :, s:e]
        lv = np.zeros((D, R), np.float32)
        lv[:, s - lo:e - lo] = lvl[b, s:e].T
        ll = np.einsum('dr,ldc->lrc', lv, woa_f) + boa[:, None, :]
        lvlog = np.zeros((NL, 96, NQ * 64), np.float32)
        for g in range(NQ):
            nq = min(QT, R - QT * g)
            lvlog[:, 0:nq, g * 64:(g + 1) * 64] = ll[:, QT * g:QT * g + nq]
        valid = np.zeros(R + 256, np.float32)
        valid[s - lo + 128:e - lo + 128] = 1.0
        vwin = np.zeros((128, NQ), np.float32)
        for g in range(NQ):
            wlo = QT * g - HB
            vwin[:, g] = valid[wlo + 128:wlo + 256]
        in_maps.append({**shared,
                        'src_sl': xs.reshape(KT, 128, RP),
                        'lvlog': lvlog,
                        'vwin': vwin})
    return in_maps


def kernel(**inputs):
    if 'nc' not in _CACHE:
        _CACHE['nc'] = build_program()
    nc = _CACHE['nc']
    in_maps = _host_prep(inputs)
    res = run_bass_kernel_spmd(nc, in_maps, list(range(8)))
    _CACHE['res'] = res
    out = np.zeros((BS, D, T), np.float32)
    for c in range(8):
        b, q4 = c // 4, c % 4
        out[b, :, 512 * q4:512 * (q4 + 1)] = \
            res.results[c]['xout'].reshape(D, 512)
    mask = np.asarray(inputs['mask'])
    return out, mask[:, None]
